# revision 1
# baseline (speedup 1.0000x reference)
"""Multi-head attention block (B=2, N=2048, C=1024, H=16, hd=64) on 8 TRN2 NeuronCores.

Sharding: data-parallel over batch (2 groups of 4 cores), tensor-parallel over
heads within each group (4 heads/core). Each core computes q/k/v for its heads,
attention, and a partial output projection; a ReduceScatter over the 4-core
group sums the partials, and the host reassembles the full [2, 2048, 1024]
output from the per-core shards.

Per-core layouts (everything transposed so the contraction dim sits on SBUF
partitions; the host pre-transposes x):
  xt   [1024, 2048]  x[b].T
  wqk  [1024, 512]   w_qkv columns for this core's q (256) ++ k (256)
  wv   [1024, 256]   w_qkv columns for this core's v
  wpb  [256, 1024]   w_proj rows for this core's heads
  bq   [128, 8]      b_proj/4, bq[p, m] = b_proj[m*128+p]/4
  out  [256, 2048]   rows g*256:(g+1)*256 of (x[b] @ ... ).T after RS
"""
import sys

if '/opt/trn_rl_repo' not in sys.path:
    sys.path.insert(0, '/opt/trn_rl_repo')

import numpy as np

import concourse.bass as bass
import concourse.mybir as mybir
import concourse.tile as tile
from concourse import bacc
from concourse.bass_utils import run_bass_kernel_spmd

F32 = mybir.dt.float32
F32R = mybir.dt.float32r
BF16 = mybir.dt.bfloat16
F16 = mybir.dt.float16

B = 2
N = 2048          # sequence length
C = 1024          # model dim
HEADS_PER_CORE = 4
HD = 64           # head dim
SCALE = HD ** -0.5
NT = N // 128     # 16 row tiles
CT = C // 128     # 8 contraction tiles
QC = 4            # q-chunks of 512
QCS = N // QC     # 512
GROUPS = [[0, 1, 2, 3], [4, 5, 6, 7]]

_NC_CACHE = None


def build():
    nc = bacc.Bacc(None, target_bir_lowering=False, debug=False)

    xt_ext = nc.declare_dram_parameter("xt", [C, N], F16, isOutput=False)
    wqk_ext = nc.declare_dram_parameter("wqk", [C, 512], F16, isOutput=False)
    wv_ext = nc.declare_dram_parameter("wv", [C, 256], F16, isOutput=False)
    wpc_ext = nc.declare_dram_parameter("wpc", [C, 256], F16, isOutput=False)
    bc_ext = nc.declare_dram_parameter("bc", [128, 2], F32, isOutput=False)
    ones_ext = nc.declare_dram_parameter("ones64", [128, 64], F16, isOutput=False)
    zeros_ext = nc.declare_dram_parameter("zeros63", [128, 63], F16, isOutput=False)
    out_ext = nc.declare_dram_parameter("out", [256, N], F16, isOutput=True)

    with tile.TileContext(nc) as tc:
        with (
            tc.tile_pool(name="weights", bufs=1) as wpool,
            tc.tile_pool(name="acts", bufs=1) as apool,
            tc.tile_pool(name="work", bufs=4) as work,
            tc.tile_pool(name="norm", bufs=2) as npool,
            tc.tile_pool(name="dram", bufs=2, space="DRAM") as dram,
            tc.tile_pool(name="dram4", bufs=4, space="DRAM") as dram4,
        ):
            # ---- load inputs ----
            wqk_sb = wpool.tile([128, CT, 512], F16, tag="wqk")
            wv_sb = wpool.tile([128, CT, 256], F16, tag="wv")
            wp_sb = wpool.tile([128, CT, 256], F16, tag="wp")
            bc_sb = wpool.tile([128, 2], F32, tag="bc")

            wqk_r = wqk_ext.ap().rearrange("(t p) n -> t p n", p=128)
            wv_r = wv_ext.ap().rearrange("(t p) n -> t p n", p=128)
            wpc_r = wpc_ext.ap().rearrange("(t p) n -> t p n", p=128)


            # ---- phase A: qkT = wqk.T @ xt   [512, 2048], v = xt.T @ wv [2048, 256+ones] ----
            qk_sb = apool.tile([128, 4, N], F16, tag="qk")
            v_sb = apool.tile([128, NT, HEADS_PER_CORE, 128], F16, tag="v")
            # ones column for the row-sum trick (memset can't write f32r tiles;
            # DMA from a host constant instead)
            nc.sync.dma_start(
                out=v_sb[:, :, :, HD:HD + 1],
                in_=ones_ext.ap().rearrange("p (a b c) -> p a b c", a=NT, b=HEADS_PER_CORE),
            )
            # cols HD+1..127 of each v block are never-read pad (FWL needs a
            # 128-col stationary); zero them so no stale NaNs enter PSUM
            nc.sync.dma_start(
                out=v_sb[:, :, :, HD + 1:128],
                in_=bass.AP(tensor=zeros_ext.ap().tensor, offset=0,
                            ap=[[63, 128], [0, NT * HEADS_PER_CORE], [1, 63]]),
            )
            with (
                tc.tile_pool(name="psA", bufs=4, space="PSUM") as psA_pool,
            ):
                xt_sb = apool.tile([128, CT, N], F16, tag="xt")
                xt_r = xt_ext.ap().rearrange("(t p) n -> t p n", p=128)
                for ct in range(CT):
                    nc.sync.dma_start(out=xt_sb[:, ct, :], in_=xt_r[ct])
                    nc.sync.dma_start(out=wqk_sb[:, ct, :], in_=wqk_r[ct])
                    nc.sync.dma_start(out=wv_sb[:, ct, :], in_=wv_r[ct])
                    nc.sync.dma_start(out=wp_sb[:, ct, :], in_=wpc_r[ct])
                nc.sync.dma_start(out=bc_sb[:, :], in_=bc_ext[:, :])
                for m in (0, 2):
                    psAs = [psA_pool.tile([128, QCS], F32, tag="psA", name=f"psA_{m}_{i}") for i in range(QC)]
                    for ct in range(CT):
                        for qn in range(QC):
                            nc.tensor.matmul(
                                psAs[qn][:, :],
                                wqk_sb[:, ct, m * 128:(m + 1) * 128],
                                xt_sb[:, ct, qn * QCS:(qn + 1) * QCS],
                                start=(ct == 0), stop=(ct == CT - 1),
                            )
                    for qn in range(QC):
                        nc.vector.tensor_copy(qk_sb[:, m, qn * QCS:(qn + 1) * QCS], psAs[qn][:, :])
                for rtc in range(NT // 4):
                    psVs = [psA_pool.tile([128, 256], F32, tag="psA", name=f"psV_{rtc}_{i}") for i in range(4)]
                    for ct in range(CT):
                        for j in range(4):
                            nc.tensor.matmul(
                                psVs[j][:, :],
                                xt_sb[:, ct, (rtc * 4 + j) * 128:(rtc * 4 + j + 1) * 128],
                                wv_sb[:, ct, :],
                                start=(ct == 0), stop=(ct == CT - 1),
                            )
                    for j in range(4):
                        nc.vector.tensor_copy(
                            v_sb[:, rtc * 4 + j, :, 0:HD],
                            psVs[j][:, :].rearrange("p (h e) -> p h e", h=HEADS_PER_CORE),
                        )

            # ---- phases B/C/D per q-chunk ----
            with (
                tc.tile_pool(name="ofpool", bufs=4) as ofpool,
                tc.tile_pool(name="psS", bufs=2, space="PSUM") as psS_pool,
                tc.tile_pool(name="psO", bufs=3, space="PSUM") as psO_pool,
                tc.tile_pool(name="psP", bufs=1, space="PSUM") as psP_pool,
            ):
                def scores(qc, pr, kt):
                    qsl_ = slice(qc * QCS, (qc + 1) * QCS)
                    ksl = slice(kt * 128, (kt + 1) * 128)
                    psS = psS_pool.tile([128, 2 * QCS], F32, tag="psS",
                                        name=f"psS_{qc}_{pr}_{kt}")
                    nc.tensor.matmul(
                        psS[:, 0:QCS],
                        qk_sb[0:64, 2 + pr, ksl],
                        qk_sb[0:64, pr, qsl_],
                        start=True, stop=True,
                    )
                    nc.tensor.matmul(
                        psS[:, QCS:2 * QCS],
                        qk_sb[64:128, 2 + pr, ksl],
                        qk_sb[64:128, pr, qsl_],
                        start=True, stop=True,
                    )
                    return psS

                ags = {}

                def do_proj(qc):
                    qsl_ = slice(qc * QCS, (qc + 1) * QCS)
                    of_sbs = []
                    for pr in range(2):
                        of_sb = ofpool.tile([128, 4, QCS], F16, tag="of", name=f"of_{qc}_{pr}")
                        ag_r = ags[(qc, pr)][:, :].rearrange("(t p) n -> t p n", p=128)
                        for t in range(4):
                            nc.sync.dma_start(out=of_sb[:, t, :], in_=ag_r[t])
                        of_sbs.append(of_sb)
                    for m2 in range(2):
                        psP = psP_pool.tile([128, QCS], F32, tag="psP", name=f"psP_{qc}_{m2}")
                        for kt8 in range(CT):
                            nc.tensor.matmul(
                                psP[:, :],
                                wp_sb[:, kt8, m2 * 128:(m2 + 1) * 128],
                                of_sbs[kt8 // 4][:, kt8 % 4, :],
                                start=(kt8 == 0), stop=(kt8 == CT - 1),
                            )
                        outsb = work.tile([128, QCS], F16, tag="outsb", name=f"outsb_{qc}_{m2}")
                        nc.vector.tensor_scalar_add(outsb[:, :], psP[:, :], bc_sb[:, m2:m2 + 1])
                        nc.sync.dma_start(out=out_ext[m2 * 128:(m2 + 1) * 128, qsl_], in_=outsb[:, :])

                qk_pending = [(1, 0), (3, 0), (3, 1), (3, 2), (3, 3), (1, 1), (1, 2), (1, 3)]

                def emit_qk_group(m, qn):
                    psq = psP_pool.tile([128, QCS], F32, tag="psP", name=f"psq_{m}_{qn}")
                    for ct in range(CT):
                        nc.tensor.matmul(
                            psq[:, :],
                            wqk_sb[:, ct, m * 128:(m + 1) * 128],
                            xt_sb[:, ct, qn * QCS:(qn + 1) * QCS],
                            start=(ct == 0), stop=(ct == CT - 1),
                        )
                    nc.vector.tensor_copy(qk_sb[:, m, qn * QCS:(qn + 1) * QCS], psq[:, :])

                blocks = [(qc, pr) for qc in range(QC) for pr in range(2)]
                psS_cur = scores(0, 0, 0)
                for bi, (qc, pr) in enumerate(blocks):
                    on_sb = npool.tile([128, QCS], F16, tag="on", name=f"on_{qc}_{pr}")
                    psO_e = psO_pool.tile([128, QCS], F32, tag="psO", name=f"psOe_{qc}_{pr}")
                    psO_o = psO_pool.tile([128, QCS], F32, tag="psO", name=f"psOo_{qc}_{pr}")
                    for kt in range(NT):
                        # 1-deep software pipeline across block boundaries: the
                        # in-order PE must never sit directly behind exp(kt)
                        if kt + 1 < NT:
                            nxt = (qc, pr, kt + 1)
                        elif bi + 1 < len(blocks):
                            nxt = (blocks[bi + 1][0], blocks[bi + 1][1], 0)
                        else:
                            nxt = None
                        psS_next = scores(*nxt) if nxt else None
                        expt = work.tile([128, 2 * QCS], F16, tag="expt",
                                         name=f"expt_{qc}_{pr}_{kt}")
                        nc.scalar.activation(
                            expt[:, :], psS_cur[:, :],
                            mybir.ActivationFunctionType.Exp,
                            bias=0.0, scale=SCALE,
                        )
                        nc.tensor.matmul(
                            psO_e[:, :],
                            v_sb[:, kt, 2 * pr, 0:128],
                            expt[:, 0:QCS],
                            start=(kt == 0), stop=(kt == NT - 1),
                        )
                        nc.tensor.matmul(
                            psO_o[:, :],
                            v_sb[:, kt, 2 * pr + 1, 0:128],
                            expt[:, QCS:2 * QCS],
                            start=(kt == 0), stop=(kt == NT - 1),
                        )
                        psS_cur = psS_next
                        if bi == 0 and kt % 2 == 0 and qk_pending:
                            emit_qk_group(*qk_pending.pop(0))
                    # normalize: o / rowsum (rowsum is psO[64], per q position).
                    # Copy PSUM->SBUF first so the PSUM slot frees before the
                    # slow [1,512] reciprocal.
                    for hh, psO in ((0, psO_e), (1, psO_o)):
                        o_sb = npool.tile([65, QCS], F32, tag="o_sb", name=f"osb_{qc}_{pr}_{hh}")
                        nc.vector.tensor_copy(o_sb[:, :], psO[0:65, :])
                        recip = npool.tile([65, QCS], F32, tag="recip", name=f"rc_{qc}_{pr}_{hh}")
                        nc.vector.reciprocal(recip[64:65, :], o_sb[64:65, :])
                        row_dram = dram.tile([1, QCS], F32, tag="row", name=f"row_{qc}_{pr}_{hh}")
                        nc.sync.dma_start(out=row_dram[:, :], in_=recip[64:65, :])
                        rd = row_dram[:, :]
                        bcast_src = bass.AP(
                            tensor=rd.tensor, offset=rd.offset,
                            ap=[[0, 64]] + list(rd.ap[1:]),
                        )
                        bcast = npool.tile([64, QCS], F32, tag="bcast", name=f"bc_{qc}_{pr}_{hh}")
                        nc.sync.dma_start(out=bcast[:, :], in_=bcast_src)
                        nc.vector.tensor_mul(
                            on_sb[hh * 64:(hh + 1) * 64, :],
                            o_sb[0:64, :],
                            bcast[:, :],
                        )
                    # all-gather this pair's attention outputs (128KB, mesh regime)
                    og = dram.tile([128, QCS], F16, tag="og", name=f"og_{qc}_{pr}")
                    nc.sync.dma_start(out=og[:, :], in_=on_sb[:, :])
                    ag = dram4.tile([512, QCS], F16, tag="ag", name=f"ag_{qc}_{pr}")
                    nc.gpsimd.collective_compute(
                        "AllGather",
                        mybir.AluOpType.bypass,
                        replica_groups=GROUPS,
                        ins=[og.opt()],
                        outs=[ag.opt()],
                    )
                    ags[(qc, pr)] = ag
                    # deferred projection: qc-1's AG completed during this qc's
                    # attention, so its matmuls never stall the in-order PE
                    if pr == 1 and qc > 0:
                        do_proj(qc - 1)
                do_proj(QC - 1)

    nc.compile()
    return nc


def _get_nc():
    global _NC_CACHE
    if _NC_CACHE is None:
        _NC_CACHE = build()
    return _NC_CACHE


def shard_inputs(x, w_qkv, w_proj, b_proj):
    x = np.asarray(x, dtype=np.float32)
    w_qkv = np.asarray(w_qkv, dtype=np.float32)
    w_proj = np.asarray(w_proj, dtype=np.float32)
    b_proj = np.asarray(b_proj, dtype=np.float32)
    # ag row order: for each pr, rank-major then local-head-major:
    # rows [j*128 + h2*64 + e] <-> global head 4j + 2*pr + h2
    perm = np.concatenate([
        np.arange(1024).reshape(16, 64)[[4 * j + 2 * pr + h2 for j in range(4) for h2 in range(2)]].reshape(-1)
        for pr in range(2)
    ])
    in_maps = []
    for core in range(8):
        b, g = divmod(core, 4)
        cs = slice(g * 256, (g + 1) * 256)
        wqk = np.concatenate([w_qkv[:, 0 * C + g * 256:0 * C + (g + 1) * 256],
                              w_qkv[:, 1 * C + g * 256:1 * C + (g + 1) * 256]], axis=1)
        in_maps.append({
            "xt": np.ascontiguousarray(x[b].T.astype(np.float16)),
            "wqk": np.ascontiguousarray(wqk.astype(np.float16)),
            "wv": np.ascontiguousarray(w_qkv[:, 2 * C + g * 256:2 * C + (g + 1) * 256].astype(np.float16)),
            "wpc": np.ascontiguousarray(w_proj[perm, :][:, cs].astype(np.float16)),
            "bc": np.ascontiguousarray(b_proj[cs].reshape(2, 128).T),
            "ones64": np.ones((128, 64), dtype=np.float16),
            "zeros63": np.zeros((128, 63), dtype=np.float16),
        })
    return in_maps


def assemble_output(results):
    outT = np.empty((B, C, N), dtype=np.float32)
    for core in range(8):
        b, g = divmod(core, 4)
        outT[b, g * 256:(g + 1) * 256, :] = np.asarray(results[core]["out"], dtype=np.float32)
    return np.ascontiguousarray(outT.transpose(0, 2, 1))


def run_sharded(x, w_qkv, w_proj, b_proj, trace=False):
    nc = _get_nc()
    in_maps = shard_inputs(x, w_qkv, w_proj, b_proj)
    res = run_bass_kernel_spmd(nc, in_maps, core_ids=list(range(8)), trace=trace)
    return assemble_output(res.results), res.exec_time_ns


def kernel(x, w_qkv, w_proj, b_proj):
    out, _ = run_sharded(x, w_qkv, w_proj, b_proj, trace=False)
    return out



# revision 10
# speedup vs baseline: 1.1071x; 1.1071x over previous
"""Multi-head attention block (B=2, N=2048, C=1024, H=16, hd=64) on 8 TRN2 NeuronCores.

Sharding: data-parallel over batch (2 groups of 4 cores), tensor-parallel over
heads within each group (4 heads/core). Each core computes q/k/v for its heads,
attention, and a partial output projection; an AllGather over the 4-core group
collects head outputs, and each core projects its 256-column slice.

v2 schedule: single fused loop. The softmax exp on the Scalar engine
(~1.3us per 1024 columns x 128 iterations) and the PE matmul stream
(~164us of column-cycles) are the two near-equal rooflines, so all QKV /
projection matmuls are streamed INTO the attention loop's PE slack instead
of running in separate phases where the other engine would idle. Blocks run
pr-major ((qc,pr0) x4 then (qc,pr1) x4) so only block 0 carries forced k/v
emissions. Input DMA is a few large transfers on both HWDGE rings (SP+ACT)
ordered so the first score matmul can start ~3us in.

Per-core layouts (contraction dim on SBUF partitions; host pre-transposes x):
  xt   [1024, 2048]  x[b].T
  wqk  [1024, 512]   w_qkv columns for this core's q (256) ++ k (256)
  wv   [1024, 256]   w_qkv columns for this core's v
  wpb  [256, 1024]   w_proj rows for this core's heads (perm'd, see host code)
  bc   [128, 2]      bc[p, m] = b_proj[g*256 + m*128 + p]
  out  [256, 2048]   rows g*256:(g+1)*256 of (x[b] @ ... ).T
"""
import sys

if '/opt/trn_rl_repo' not in sys.path:
    sys.path.insert(0, '/opt/trn_rl_repo')

import numpy as np

import concourse.bass as bass
import concourse.mybir as mybir
import concourse.tile as tile
from concourse import bacc
from concourse.bass_utils import run_bass_kernel_spmd

F32 = mybir.dt.float32
F16 = mybir.dt.float16

B = 2
N = 2048          # sequence length
C = 1024          # model dim
HD = 64           # head dim
SCALE = HD ** -0.5
NT = N // 128     # 16 key tiles
CT = C // 128     # 8 contraction tiles
QC = 4            # q-chunks of 512
QCS = N // QC     # 512
GROUPS = [[0, 1, 2, 3], [4, 5, 6, 7]]
NITER = 8 * NT    # 8 blocks x 16 key tiles

_NC_CACHE = None


def _blk(b):
    """pr-major block order: blocks 0-3 = (qc, pr0), 4-7 = (qc, pr1)."""
    return (b % 4, b // 4)  # (qc, pr)


def build():
    nc = bacc.Bacc(None, target_bir_lowering=False, debug=False)

    xt_ext = nc.declare_dram_parameter("xt", [C, N], F16, isOutput=False)
    wqk_ext = nc.declare_dram_parameter("wqk", [C, 512], F16, isOutput=False)
    wv_ext = nc.declare_dram_parameter("wv", [C, 256], F16, isOutput=False)
    wpc_ext = nc.declare_dram_parameter("wpc", [C, 256], F16, isOutput=False)
    bc_ext = nc.declare_dram_parameter("bc", [128, 2], F32, isOutput=False)
    ones_ext = nc.declare_dram_parameter("ones64", [128, 64], F16, isOutput=False)
    zeros_ext = nc.declare_dram_parameter("zeros63", [128, 63], F16, isOutput=False)
    out_ext = nc.declare_dram_parameter("out", [256, N], F16, isOutput=True)

    with tile.TileContext(nc) as tc:
        with (
            tc.tile_pool(name="weights", bufs=1) as wpool,
            tc.tile_pool(name="acts", bufs=1) as apool,
            tc.tile_pool(name="expt", bufs=3) as epool,
            tc.tile_pool(name="norm", bufs=2) as npool,
            tc.tile_pool(name="outp", bufs=2) as opool,
            tc.tile_pool(name="ofp", bufs=2) as ofpool,
            tc.tile_pool(name="psS", bufs=2, space="PSUM") as psS_pool,
            tc.tile_pool(name="psE", bufs=2, space="PSUM") as psE_pool,
            tc.tile_pool(name="psO", bufs=1, space="PSUM") as psO_pool,
            tc.tile_pool(name="dramog", bufs=2, space="DRAM") as og_pool,
            tc.tile_pool(name="dramag", bufs=8, space="DRAM") as ag_pool,
        ):
            # ---- SBUF tiles ----
            xt_sb = apool.tile([128, CT, N], F16, tag="xt")
            wqk_sb = wpool.tile([128, CT, 512], F16, tag="wqk")
            wv_sb = wpool.tile([128, CT, 256], F16, tag="wv")
            wp_sb = wpool.tile([128, CT, 256], F16, tag="wp")
            bc_sb = wpool.tile([128, 2], F32, tag="bc")
            ones_row = wpool.tile([1, 64], F16, tag="ones_row")
            qk_sb = apool.tile([128, 4, N], F16, tag="qk")
            v_sb = apool.tile([128, NT, 4, 128], F16, tag="v")

            # preload the Exp activation table while input DMAs stream
            dmy = npool.tile([1, 8], F32, tag="dmy")
            nc.vector.memset(dmy[:, :], 0.0)
            dmy2 = npool.tile([1, 8], F16, tag="dmy2")
            nc.scalar.activation(dmy2[:, :], dmy[:, :],
                                 mybir.ActivationFunctionType.Exp,
                                 bias=0.0, scale=1.0)

            # in-side APs permuted to [p][t][n] to match SBUF tile iteration order
            xt_r = xt_ext.ap().rearrange("(t p) n -> p t n", p=128)
            wqk_r = wqk_ext.ap().rearrange("(t p) n -> p t n", p=128)
            wv_r = wv_ext.ap().rearrange("(t p) n -> p t n", p=128)
            wpc_r = wpc_ext.ap().rearrange("(t p) n -> p t n", p=128)

            # ---- input DMAs: big transfers, two HWDGE rings, critical first.
            # SP ring: what the first k/q emissions need, then the xt stream.
            nc.sync.dma_start(out=wqk_sb[:, :, 256:384], in_=wqk_r[:, :, 256:384])
            nc.sync.dma_start(out=xt_sb[:, :, 0:128], in_=xt_r[:, :, 0:128])
            nc.sync.dma_start(out=wqk_sb[:, :, 0:128], in_=wqk_r[:, :, 0:128])
            nc.sync.dma_start(out=xt_sb[:, :, 128:512], in_=xt_r[:, :, 128:512])
            nc.sync.dma_start(out=xt_sb[:, :, 512:1024], in_=xt_r[:, :, 512:1024])
            nc.sync.dma_start(out=xt_sb[:, :, 1024:1536], in_=xt_r[:, :, 1024:1536])
            nc.sync.dma_start(out=xt_sb[:, :, 1536:2048], in_=xt_r[:, :, 1536:2048])
            # ACT ring (parallel with SP): v weights + the rest
            nc.scalar.dma_start(out=wv_sb[:, :, :], in_=wv_r[:, :, :])
            nc.scalar.dma_start(
                out=v_sb[:, :, :, HD:HD + 1],
                in_=ones_ext.ap().rearrange("p (a b c) -> p a b c", a=NT, b=4),
            )
            nc.scalar.dma_start(
                out=v_sb[:, :, :, HD + 1:128],
                in_=bass.AP(tensor=zeros_ext.ap().tensor, offset=0,
                            ap=[[63, 128], [0, NT * 4], [1, 63]]),
            )
            nc.scalar.dma_start(out=ones_row[:, :], in_=ones_ext[0:1, 0:64])
            nc.scalar.dma_start(out=bc_sb[:, :], in_=bc_ext[:, :])
            nc.scalar.dma_start(out=wqk_sb[:, :, 128:256], in_=wqk_r[:, :, 128:256])
            nc.scalar.dma_start(out=wqk_sb[:, :, 384:512], in_=wqk_r[:, :, 384:512])
            nc.scalar.dma_start(out=wp_sb[:, :, :], in_=wpc_r[:, :, :])

            # ---- emission helpers (PE work streamed into the loop) ----
            def emit_k(pr, kt):
                ksl = slice(kt * 128, (kt + 1) * 128)
                psq = psE_pool.tile([128, 128], F32, tag="psE", name=f"psk_{pr}_{kt}")
                for ct in range(CT):
                    nc.tensor.matmul(
                        psq[:, :],
                        wqk_sb[:, ct, 256 + pr * 128:384 + pr * 128],
                        xt_sb[:, ct, ksl],
                        start=(ct == 0), stop=(ct == CT - 1),
                    )
                nc.vector.tensor_copy(qk_sb[:, 2 + pr, ksl], psq[:, :])

            def emit_q(pr, qc):
                qsl = slice(qc * QCS, (qc + 1) * QCS)
                psq = psE_pool.tile([128, QCS], F32, tag="psE", name=f"psq_{pr}_{qc}")
                for ct in range(CT):
                    nc.tensor.matmul(
                        psq[:, :],
                        wqk_sb[:, ct, pr * 128:(pr + 1) * 128],
                        xt_sb[:, ct, qsl],
                        start=(ct == 0), stop=(ct == CT - 1),
                    )
                nc.vector.tensor_copy(qk_sb[:, pr, qsl], psq[:, :])

            def emit_v(kt):
                ksl = slice(kt * 128, (kt + 1) * 128)
                psv = psE_pool.tile([128, 256], F32, tag="psE", name=f"psv_{kt}")
                for ct in range(CT):
                    nc.tensor.matmul(
                        psv[:, :],
                        xt_sb[:, ct, ksl],
                        wv_sb[:, ct, :],
                        start=(ct == 0), stop=(ct == CT - 1),
                    )
                nc.vector.tensor_copy(
                    v_sb[:, kt, :, 0:HD],
                    psv[:, :].rearrange("p (h e) -> p h e", h=4),
                )

            def scores(b, kt):
                qc, pr = _blk(b)
                qsl = slice(qc * QCS, (qc + 1) * QCS)
                ksl = slice(kt * 128, (kt + 1) * 128)
                psS = psS_pool.tile([128, 2 * QCS], F32, tag="psS",
                                    name=f"psS_{b}_{kt}")
                nc.tensor.matmul(
                    psS[:, 0:QCS],
                    qk_sb[0:64, 2 + pr, ksl],
                    qk_sb[0:64, pr, qsl],
                    start=True, stop=True,
                )
                nc.tensor.matmul(
                    psS[:, QCS:2 * QCS],
                    qk_sb[64:128, 2 + pr, ksl],
                    qk_sb[64:128, pr, qsl],
                    start=True, stop=True,
                )
                return psS

            # ---- per-block normalize / gather / project ----
            norm_state = {}
            ags = {}

            def part1(b):
                """Drain psO: o and rowsums to SBUF (frees psO for next block)."""
                psO = norm_state.pop(('psO', b))
                o2 = npool.tile([128, QCS], F32, tag="o2", name=f"o2_{b}")
                rs_e = npool.tile([1, QCS], F32, tag="rs_e", name=f"rse_{b}")
                rs_o = npool.tile([1, QCS], F32, tag="rs_o", name=f"rso_{b}")
                # bank A (head e) first so next block's first PV can start early
                nc.vector.tensor_copy(o2[0:64, :], psO[0:64, 0:QCS])
                nc.vector.tensor_copy(rs_e[:, :], psO[64:65, 0:QCS])
                nc.vector.tensor_copy(o2[64:128, :], psO[0:64, QCS:2 * QCS])
                nc.vector.tensor_copy(rs_o[:, :], psO[64:65, QCS:2 * QCS])
                norm_state[('o2', b)] = o2
                norm_state[('rs', b)] = (rs_e, rs_o)

            def part2(b):
                """1/rowsum (fast approx), broadcast via PE matmul, normalize."""
                o2 = norm_state.pop(('o2', b))
                rs_e, rs_o = norm_state.pop(('rs', b))
                psB = psE_pool.tile([128, QCS], F32, tag="psE", name=f"psB_{b}")
                for hh, rs in ((0, rs_e), (1, rs_o)):
                    rcf = npool.tile([1, QCS], F32, tag="rcf", name=f"rcf_{b}_{hh}")
                    nc.vector.reciprocal_approx_fast(out=rcf[:, :], in_=rs[:, :])
                    rc16 = npool.tile([1, QCS], F16, tag="rc16", name=f"rc16_{b}_{hh}")
                    nc.vector.tensor_copy(rc16[:, :], rcf[:, :])
                    nc.tensor.matmul(psB[hh * 64:(hh + 1) * 64, :],
                                     ones_row[:, :], rc16[:, :],
                                     start=True, stop=True)
                on_sb = npool.tile([128, QCS], F16, tag="on", name=f"on_{b}")
                nc.vector.tensor_mul(on_sb[:, :], o2[:, :], psB[:, :])
                norm_state[('on', b)] = on_sb

            def part3(b):
                """Store + AllGather this block's head outputs."""
                qc, pr = _blk(b)
                on_sb = norm_state.pop(('on', b))
                og = og_pool.tile([128, QCS], F16, tag="og", name=f"og_{b}")
                nc.sync.dma_start(out=og[:, :], in_=on_sb[:, :])
                ag = ag_pool.tile([512, QCS], F16, tag="ag", name=f"ag_{b}")
                nc.gpsimd.collective_compute(
                    "AllGather",
                    mybir.AluOpType.bypass,
                    replica_groups=GROUPS,
                    ins=[og.opt()],
                    outs=[ag.opt()],
                )
                ags[(qc, pr)] = ag
                if pr == 1:
                    emit_ofload(qc)

            of_sbs = {}

            def emit_ofload(qc):
                of_sb = ofpool.tile([128, CT, QCS], F16, tag="of", name=f"of_{qc}")
                for pr in range(2):
                    ag_r = ags[(qc, pr)][:, :].rearrange("(t p) n -> p t n", p=128)
                    nc.sync.dma_start(out=of_sb[:, pr * 4:(pr + 1) * 4, :], in_=ag_r)
                of_sbs[qc] = of_sb

            def emit_proj(qc, m2):
                qsl = slice(qc * QCS, (qc + 1) * QCS)
                of_sb = of_sbs[qc]
                psP = psE_pool.tile([128, QCS], F32, tag="psE", name=f"psP_{qc}_{m2}")
                for t in range(CT):
                    nc.tensor.matmul(
                        psP[:, :],
                        wp_sb[:, t, m2 * 128:(m2 + 1) * 128],
                        of_sb[:, t, :],
                        start=(t == 0), stop=(t == CT - 1),
                    )
                outsb = opool.tile([128, QCS], F16, tag="outsb",
                                   name=f"outsb_{qc}_{m2}")
                nc.vector.tensor_scalar_add(outsb[:, :], psP[:, :],
                                            bc_sb[:, m2:m2 + 1])
                nc.sync.dma_start(out=out_ext[m2 * 128:(m2 + 1) * 128, qsl],
                                  in_=outsb[:, :])

            # ---- static emission schedule: iter -> list of thunks ----
            sched = {}

            def at(i, fn):
                sched.setdefault(i, []).append(fn)

            for kt in range(2, NT):               # k(pr0) streamed in block 0
                at(kt - 2, lambda kt=kt: emit_k(0, kt))
            for kt in range(1, NT):               # v streamed in block 0
                at(kt - 1, lambda kt=kt: emit_v(kt))
            at(10, lambda: emit_q(0, 1))          # due block 1 (iter 15)
            at(21, lambda: emit_q(0, 2))          # due block 2 (iter 31)
            at(37, lambda: emit_q(0, 3))          # due block 3 (iter 47)
            at(45, lambda: emit_q(1, 0))          # due block 4 (iter 63)
            at(61, lambda: emit_q(1, 1))          # due block 5 (iter 79)
            at(77, lambda: emit_q(1, 2))          # due block 6 (iter 95)
            at(93, lambda: emit_q(1, 3))          # due block 7 (iter 111)
            for kt in range(NT):                  # k(pr1) due block 4 (iter 64+kt)
                at(32 + 2 * kt, lambda kt=kt: emit_k(1, kt))
            for qc in range(3):                   # proj(qc) after AG(qc,pr1)
                at(16 * (4 + qc) + 24, lambda qc=qc: emit_proj(qc, 0))
                at(16 * (4 + qc) + 26, lambda qc=qc: emit_proj(qc, 1))
            for b in range(7):                    # normalize tail of each block
                at(16 * b + 17, lambda b=b: part2(b))
                at(16 * b + 18, lambda b=b: part3(b))

            # ---- prologue ----
            emit_k(0, 0)
            emit_v(0)
            emit_q(0, 0)
            emit_k(0, 1)
            psS_cur = scores(0, 0)

            # ---- main loop ----
            for bi in range(NITER):
                b, kt = bi // NT, bi % NT
                qc, pr = _blk(b)
                psS_next = scores(bi // NT if kt < NT - 1 else b + 1,
                                  (kt + 1) % NT) if bi + 1 < NITER else None
                expt = epool.tile([128, 2 * QCS], F16, tag="expt",
                                  name=f"expt_{bi}")
                nc.scalar.activation(
                    expt[:, :], psS_cur[:, :],
                    mybir.ActivationFunctionType.Exp,
                    bias=0.0, scale=SCALE,
                )
                for fn in sched.get(bi, ()):
                    fn()
                if kt == 0:
                    norm_state[('psO', b)] = psO_pool.tile(
                        [128, 2 * QCS], F32, tag="psO", name=f"psO_{b}")
                psO = norm_state[('psO', b)]
                nc.tensor.matmul(
                    psO[:, 0:QCS],
                    v_sb[:, kt, 2 * pr, 0:128],
                    expt[:, 0:QCS],
                    start=(kt == 0), stop=(kt == NT - 1),
                )
                nc.tensor.matmul(
                    psO[:, QCS:2 * QCS],
                    v_sb[:, kt, 2 * pr + 1, 0:128],
                    expt[:, QCS:2 * QCS],
                    start=(kt == 0), stop=(kt == NT - 1),
                )
                psS_cur = psS_next
                if kt == NT - 1:
                    part1(b)

            # ---- tail: last block's normalize + gather + projection ----
            part2(7)
            part3(7)
            emit_proj(3, 0)
            emit_proj(3, 1)

    nc.compile()
    return nc


def _get_nc():
    global _NC_CACHE
    if _NC_CACHE is None:
        _NC_CACHE = build()
    return _NC_CACHE


def shard_inputs(x, w_qkv, w_proj, b_proj):
    x = np.asarray(x, dtype=np.float32)
    w_qkv = np.asarray(w_qkv, dtype=np.float32)
    w_proj = np.asarray(w_proj, dtype=np.float32)
    b_proj = np.asarray(b_proj, dtype=np.float32)
    # ag row order: for each pr, rank-major then local-head-major:
    # rows [j*128 + h2*64 + e] <-> global head 4j + 2*pr + h2
    perm = np.concatenate([
        np.arange(1024).reshape(16, 64)[[4 * j + 2 * pr + h2 for j in range(4) for h2 in range(2)]].reshape(-1)
        for pr in range(2)
    ])
    in_maps = []
    for core in range(8):
        b, g = divmod(core, 4)
        cs = slice(g * 256, (g + 1) * 256)
        wqk = np.concatenate([w_qkv[:, 0 * C + g * 256:0 * C + (g + 1) * 256],
                              w_qkv[:, 1 * C + g * 256:1 * C + (g + 1) * 256]], axis=1)
        in_maps.append({
            "xt": np.ascontiguousarray(x[b].T.astype(np.float16)),
            "wqk": np.ascontiguousarray(wqk.astype(np.float16)),
            "wv": np.ascontiguousarray(w_qkv[:, 2 * C + g * 256:2 * C + (g + 1) * 256].astype(np.float16)),
            "wpc": np.ascontiguousarray(w_proj[perm, :][:, cs].astype(np.float16)),
            "bc": np.ascontiguousarray(b_proj[cs].reshape(2, 128).T),
            "ones64": np.ones((128, 64), dtype=np.float16),
            "zeros63": np.zeros((128, 63), dtype=np.float16),
        })
    return in_maps


def assemble_output(results):
    outT = np.empty((B, C, N), dtype=np.float32)
    for core in range(8):
        b, g = divmod(core, 4)
        outT[b, g * 256:(g + 1) * 256, :] = np.asarray(results[core]["out"], dtype=np.float32)
    return np.ascontiguousarray(outT.transpose(0, 2, 1))


def run_sharded(x, w_qkv, w_proj, b_proj, trace=False):
    nc = _get_nc()
    in_maps = shard_inputs(x, w_qkv, w_proj, b_proj)
    res = run_bass_kernel_spmd(nc, in_maps, core_ids=list(range(8)), trace=trace)
    return assemble_output(res.results), res.exec_time_ns


def kernel(x, w_qkv, w_proj, b_proj):
    out, _ = run_sharded(x, w_qkv, w_proj, b_proj, trace=False)
    return out


# revision 14
# speedup vs baseline: 1.1446x; 1.0339x over previous
"""Multi-head attention block (B=2, N=2048, C=1024, H=16, hd=64) on 8 TRN2 NeuronCores.

Sharding: data-parallel over batch (2 groups of 4 cores), tensor-parallel over
heads within each group (4 heads/core). Each core computes q/k/v for its heads,
attention, and a partial output projection; an AllGather over the 4-core group
collects head outputs, and each core projects its 256-column slice.

v2 schedule: single fused loop. The softmax exp on the Scalar engine
(~1.3us per 1024 columns x 128 iterations) and the PE matmul stream
(~164us of column-cycles) are the two near-equal rooflines, so all QKV /
projection matmuls are streamed INTO the attention loop's PE slack instead
of running in separate phases where the other engine would idle. Blocks run
pr-major ((qc,pr0) x4 then (qc,pr1) x4) so only block 0 carries forced k/v
emissions. Input DMA is a few large transfers on both HWDGE rings (SP+ACT)
ordered so the first score matmul can start ~3us in.

Per-core layouts (contraction dim on SBUF partitions; host pre-transposes x):
  xt   [1024, 2048]  x[b].T
  wqk  [1024, 512]   w_qkv columns for this core's q (256) ++ k (256)
  wv   [1024, 256]   w_qkv columns for this core's v
  wpb  [256, 1024]   w_proj rows for this core's heads (perm'd, see host code)
  bc   [128, 2]      bc[p, m] = b_proj[g*256 + m*128 + p]
  out  [256, 2048]   rows g*256:(g+1)*256 of (x[b] @ ... ).T
"""
import sys

if '/opt/trn_rl_repo' not in sys.path:
    sys.path.insert(0, '/opt/trn_rl_repo')

import numpy as np

import concourse.bass as bass
import concourse.mybir as mybir
import concourse.tile as tile
from concourse import bacc
from concourse.bass_utils import run_bass_kernel_spmd

F32 = mybir.dt.float32
F16 = mybir.dt.float16

B = 2
N = 2048          # sequence length
C = 1024          # model dim
HD = 64           # head dim
SCALE = HD ** -0.5
NT = N // 128     # 16 key tiles
CT = C // 128     # 8 contraction tiles
QC = 4            # q-chunks of 512
QCS = N // QC     # 512
GROUPS = [[0, 1, 2, 3], [4, 5, 6, 7]]
NITER = 8 * NT    # 8 blocks x 16 key tiles

_NC_CACHE = None


def _blk(b):
    """pr-major block order: blocks 0-3 = (qc, pr0), 4-7 = (qc, pr1)."""
    return (b % 4, b // 4)  # (qc, pr)


def build():
    nc = bacc.Bacc(None, target_bir_lowering=False, debug=False)

    # p-major host layouts: every input DMA moves ~128 multi-KB descriptors
    # (HWDGE issue time scales with descriptor count)
    xt_ext = nc.declare_dram_parameter("xt", [QC, 128, CT, QCS], F16, isOutput=False)
    wqk_ext = nc.declare_dram_parameter("wqk", [4, 128, CT, 128], F16, isOutput=False)
    wv_ext = nc.declare_dram_parameter("wv", [128, CT, 256], F16, isOutput=False)
    wpc_ext = nc.declare_dram_parameter("wpc", [128, CT, 256], F16, isOutput=False)
    bc_ext = nc.declare_dram_parameter("bc", [128, 2], F32, isOutput=False)
    out_ext = nc.declare_dram_parameter("out", [256, N], F16, isOutput=True)

    with tile.TileContext(nc) as tc:
        with (
            tc.tile_pool(name="weights", bufs=1) as wpool,
            tc.tile_pool(name="acts", bufs=1) as apool,
            tc.tile_pool(name="expt", bufs=3) as epool,
            tc.tile_pool(name="norm", bufs=2) as npool,
            tc.tile_pool(name="outp", bufs=2) as opool,
            tc.tile_pool(name="ofp", bufs=2) as ofpool,
            tc.tile_pool(name="psS", bufs=2, space="PSUM") as psS_pool,
            tc.tile_pool(name="psE", bufs=2, space="PSUM") as psE_pool,
            tc.tile_pool(name="psO", bufs=1, space="PSUM") as psO_pool,
            tc.tile_pool(name="dramog", bufs=2, space="DRAM") as og_pool,
            tc.tile_pool(name="dramag", bufs=8, space="DRAM") as ag_pool,
        ):
            # ---- SBUF tiles ----
            # xt/wqk chunk-major so each input DMA lands in a contiguous
            # per-partition region (large descriptors)
            xt_sb = apool.tile([128, QC, CT, QCS], F16, tag="xt")
            wqk_sb = wpool.tile([128, 4, CT, 128], F16, tag="wqk")
            wv_sb = wpool.tile([128, CT, 256], F16, tag="wv")
            wp_sb = wpool.tile([128, CT, 256], F16, tag="wp")
            bc_sb = wpool.tile([128, 2], F32, tag="bc")
            ones_row = wpool.tile([1, 64], F16, tag="ones_row")
            qk_sb = apool.tile([128, 4, N], F16, tag="qk")
            v_sb = apool.tile([128, NT, 4, 128], F16, tag="v")

            # constant fills on DVE (a DMA would be thousands of descriptors)
            nc.vector.memset(ones_row[:, :], 1.0)
            nc.vector.memset(v_sb[:, :, :, HD:HD + 1], 1.0)
            nc.vector.memset(v_sb[:, :, :, HD + 1:128], 0.0)

            # preload the Exp activation table while input DMAs stream
            dmy = npool.tile([1, 8], F32, tag="dmy")
            nc.vector.memset(dmy[:, :], 0.0)
            dmy2 = npool.tile([1, 8], F16, tag="dmy2")
            nc.scalar.activation(dmy2[:, :], dmy[:, :],
                                 mybir.ActivationFunctionType.Exp,
                                 bias=0.0, scale=1.0)

            # ---- input DMAs: one per chunk, critical-path first, all on the
            # SP HWDGE ring (the ACT sequencer must stay free for exp).
            # wqk blocks: 0 = q pr0, 1 = q pr1, 2 = k pr0, 3 = k pr1.
            nc.sync.dma_start(out=wqk_sb[:, 2, :, :], in_=wqk_ext.ap()[2])
            nc.sync.dma_start(out=xt_sb[:, 0, :, :], in_=xt_ext.ap()[0])
            nc.sync.dma_start(out=wqk_sb[:, 0, :, :], in_=wqk_ext.ap()[0])
            nc.sync.dma_start(out=wv_sb[:, :, :], in_=wv_ext.ap())
            nc.sync.dma_start(out=xt_sb[:, 1, :, :], in_=xt_ext.ap()[1])
            nc.sync.dma_start(out=xt_sb[:, 2, :, :], in_=xt_ext.ap()[2])
            nc.sync.dma_start(out=xt_sb[:, 3, :, :], in_=xt_ext.ap()[3])
            nc.sync.dma_start(out=wqk_sb[:, 1, :, :], in_=wqk_ext.ap()[1])
            nc.sync.dma_start(out=wqk_sb[:, 3, :, :], in_=wqk_ext.ap()[3])
            nc.sync.dma_start(out=wp_sb[:, :, :], in_=wpc_ext.ap())
            nc.sync.dma_start(out=bc_sb[:, :], in_=bc_ext[:, :])

            # ---- emission helpers (PE work streamed into the loop) ----
            def xtcol(ct, kt):
                off = (kt % 4) * 128
                return xt_sb[:, kt // 4, ct, off:off + 128]

            def emit_k(pr, kt):
                ksl = slice(kt * 128, (kt + 1) * 128)
                psq = psE_pool.tile([128, 128], F32, tag="psE", name=f"psk_{pr}_{kt}")
                for ct in range(CT):
                    nc.tensor.matmul(
                        psq[:, :],
                        wqk_sb[:, 2 + pr, ct, :],
                        xtcol(ct, kt),
                        start=(ct == 0), stop=(ct == CT - 1),
                    )
                nc.vector.tensor_copy(qk_sb[:, 2 + pr, ksl], psq[:, :])

            def emit_q(pr, qc):
                qsl = slice(qc * QCS, (qc + 1) * QCS)
                psq = psE_pool.tile([128, QCS], F32, tag="psE", name=f"psq_{pr}_{qc}")
                for ct in range(CT):
                    nc.tensor.matmul(
                        psq[:, :],
                        wqk_sb[:, pr, ct, :],
                        xt_sb[:, qc, ct, :],
                        start=(ct == 0), stop=(ct == CT - 1),
                    )
                nc.vector.tensor_copy(qk_sb[:, pr, qsl], psq[:, :])

            def emit_v(kt):
                psv = psE_pool.tile([128, 256], F32, tag="psE", name=f"psv_{kt}")
                for ct in range(CT):
                    nc.tensor.matmul(
                        psv[:, :],
                        xtcol(ct, kt),
                        wv_sb[:, ct, :],
                        start=(ct == 0), stop=(ct == CT - 1),
                    )
                nc.vector.tensor_copy(
                    v_sb[:, kt, :, 0:HD],
                    psv[:, :].rearrange("p (h e) -> p h e", h=4),
                )

            def scores(b, kt):
                qc, pr = _blk(b)
                qsl = slice(qc * QCS, (qc + 1) * QCS)
                ksl = slice(kt * 128, (kt + 1) * 128)
                psS = psS_pool.tile([128, 2 * QCS], F32, tag="psS",
                                    name=f"psS_{b}_{kt}")
                nc.tensor.matmul(
                    psS[:, 0:QCS],
                    qk_sb[0:64, 2 + pr, ksl],
                    qk_sb[0:64, pr, qsl],
                    start=True, stop=True,
                )
                nc.tensor.matmul(
                    psS[:, QCS:2 * QCS],
                    qk_sb[64:128, 2 + pr, ksl],
                    qk_sb[64:128, pr, qsl],
                    start=True, stop=True,
                )
                return psS

            # ---- per-block normalize / gather / project ----
            norm_state = {}
            ags = {}

            def part1(b):
                """Drain psO: o and rowsums to SBUF (frees psO for next block)."""
                psO = norm_state.pop(('psO', b))
                o2 = npool.tile([128, QCS], F32, tag="o2", name=f"o2_{b}")
                rs_e = npool.tile([1, QCS], F32, tag="rs_e", name=f"rse_{b}")
                rs_o = npool.tile([1, QCS], F32, tag="rs_o", name=f"rso_{b}")
                # bank A (head e) first so next block's first PV can start early
                nc.vector.tensor_copy(o2[0:64, :], psO[0:64, 0:QCS])
                nc.vector.tensor_copy(rs_e[:, :], psO[64:65, 0:QCS])
                nc.vector.tensor_copy(o2[64:128, :], psO[0:64, QCS:2 * QCS])
                nc.vector.tensor_copy(rs_o[:, :], psO[64:65, QCS:2 * QCS])
                norm_state[('o2', b)] = o2
                norm_state[('rs', b)] = (rs_e, rs_o)

            def part2(b):
                """1/rowsum (fast approx), broadcast via PE matmul, normalize."""
                o2 = norm_state.pop(('o2', b))
                rs_e, rs_o = norm_state.pop(('rs', b))
                psB = psE_pool.tile([128, QCS], F32, tag="psE", name=f"psB_{b}")
                for hh, rs in ((0, rs_e), (1, rs_o)):
                    rcf = npool.tile([1, QCS], F32, tag="rcf", name=f"rcf_{b}_{hh}")
                    nc.vector.reciprocal_approx_fast(out=rcf[:, :], in_=rs[:, :])
                    rc16 = npool.tile([1, QCS], F16, tag="rc16", name=f"rc16_{b}_{hh}")
                    nc.vector.tensor_copy(rc16[:, :], rcf[:, :])
                    nc.tensor.matmul(psB[hh * 64:(hh + 1) * 64, :],
                                     ones_row[:, :], rc16[:, :],
                                     start=True, stop=True)
                on_sb = npool.tile([128, QCS], F16, tag="on", name=f"on_{b}")
                nc.vector.tensor_mul(on_sb[:, :], o2[:, :], psB[:, :])
                norm_state[('on', b)] = on_sb

            def part3(b):
                """Store + AllGather this block's head outputs."""
                qc, pr = _blk(b)
                on_sb = norm_state.pop(('on', b))
                og = og_pool.tile([128, QCS], F16, tag="og", name=f"og_{b}")
                nc.sync.dma_start(out=og[:, :], in_=on_sb[:, :])
                ag = ag_pool.tile([512, QCS], F16, tag="ag", name=f"ag_{b}")
                nc.gpsimd.collective_compute(
                    "AllGather",
                    mybir.AluOpType.bypass,
                    replica_groups=GROUPS,
                    ins=[og.opt()],
                    outs=[ag.opt()],
                )
                ags[(qc, pr)] = ag
                if pr == 1:
                    emit_ofload(qc)

            of_sbs = {}

            def emit_ofload(qc):
                of_sb = ofpool.tile([128, CT, QCS], F16, tag="of", name=f"of_{qc}")
                for pr in range(2):
                    ag_r = ags[(qc, pr)][:, :].rearrange("(t p) n -> p t n", p=128)
                    nc.sync.dma_start(out=of_sb[:, pr * 4:(pr + 1) * 4, :], in_=ag_r)
                of_sbs[qc] = of_sb

            def emit_proj(qc, m2):
                qsl = slice(qc * QCS, (qc + 1) * QCS)
                of_sb = of_sbs[qc]
                psP = psE_pool.tile([128, QCS], F32, tag="psE", name=f"psP_{qc}_{m2}")
                for t in range(CT):
                    nc.tensor.matmul(
                        psP[:, :],
                        wp_sb[:, t, m2 * 128:(m2 + 1) * 128],
                        of_sb[:, t, :],
                        start=(t == 0), stop=(t == CT - 1),
                    )
                outsb = opool.tile([128, QCS], F16, tag="outsb",
                                   name=f"outsb_{qc}_{m2}")
                nc.vector.tensor_scalar_add(outsb[:, :], psP[:, :],
                                            bc_sb[:, m2:m2 + 1])
                nc.sync.dma_start(out=out_ext[m2 * 128:(m2 + 1) * 128, qsl],
                                  in_=outsb[:, :])

            # ---- static emission schedule: iter -> list of thunks ----
            sched = {}

            def at(i, fn):
                sched.setdefault(i, []).append(fn)

            for kt in range(2, NT):               # k(pr0) streamed in block 0
                at(kt - 2, lambda kt=kt: emit_k(0, kt))
            for kt in range(1, NT):               # v streamed in block 0
                at(kt - 1, lambda kt=kt: emit_v(kt))
            at(10, lambda: emit_q(0, 1))          # due block 1 (iter 15)
            at(21, lambda: emit_q(0, 2))          # due block 2 (iter 31)
            at(37, lambda: emit_q(0, 3))          # due block 3 (iter 47)
            at(45, lambda: emit_q(1, 0))          # due block 4 (iter 63)
            at(61, lambda: emit_q(1, 1))          # due block 5 (iter 79)
            at(77, lambda: emit_q(1, 2))          # due block 6 (iter 95)
            at(93, lambda: emit_q(1, 3))          # due block 7 (iter 111)
            for kt in range(NT):                  # k(pr1) due block 4 (iter 64+kt)
                at(32 + 2 * kt, lambda kt=kt: emit_k(1, kt))
            for qc in range(3):                   # proj(qc) after AG(qc,pr1)
                at(16 * (4 + qc) + 24, lambda qc=qc: emit_proj(qc, 0))
                at(16 * (4 + qc) + 26, lambda qc=qc: emit_proj(qc, 1))
            for b in range(7):                    # normalize tail of each block
                at(16 * b + 17, lambda b=b: part2(b))
                at(16 * b + 18, lambda b=b: part3(b))

            # ---- prologue ----
            emit_k(0, 0)
            emit_v(0)
            emit_q(0, 0)
            emit_k(0, 1)
            psS_cur = scores(0, 0)

            # ---- main loop ----
            for bi in range(NITER):
                b, kt = bi // NT, bi % NT
                qc, pr = _blk(b)
                psS_next = scores(bi // NT if kt < NT - 1 else b + 1,
                                  (kt + 1) % NT) if bi + 1 < NITER else None
                expt = epool.tile([128, 2 * QCS], F16, tag="expt",
                                  name=f"expt_{bi}")
                nc.scalar.activation(
                    expt[:, :], psS_cur[:, :],
                    mybir.ActivationFunctionType.Exp,
                    bias=0.0, scale=SCALE,
                )
                for fn in sched.get(bi, ()):
                    fn()
                if kt == 0:
                    norm_state[('psO', b)] = psO_pool.tile(
                        [128, 2 * QCS], F32, tag="psO", name=f"psO_{b}")
                psO = norm_state[('psO', b)]
                nc.tensor.matmul(
                    psO[:, 0:QCS],
                    v_sb[:, kt, 2 * pr, 0:128],
                    expt[:, 0:QCS],
                    start=(kt == 0), stop=(kt == NT - 1),
                )
                nc.tensor.matmul(
                    psO[:, QCS:2 * QCS],
                    v_sb[:, kt, 2 * pr + 1, 0:128],
                    expt[:, QCS:2 * QCS],
                    start=(kt == 0), stop=(kt == NT - 1),
                )
                psS_cur = psS_next
                if kt == NT - 1:
                    part1(b)

            # ---- tail: last block's normalize + gather + projection ----
            part2(7)
            part3(7)
            emit_proj(3, 0)
            emit_proj(3, 1)

    nc.compile()
    return nc


def _get_nc():
    global _NC_CACHE
    if _NC_CACHE is None:
        _NC_CACHE = build()
    return _NC_CACHE


def shard_inputs(x, w_qkv, w_proj, b_proj):
    x = np.asarray(x, dtype=np.float32)
    w_qkv = np.asarray(w_qkv, dtype=np.float32)
    w_proj = np.asarray(w_proj, dtype=np.float32)
    b_proj = np.asarray(b_proj, dtype=np.float32)
    # ag row order: for each pr, rank-major then local-head-major:
    # rows [j*128 + h2*64 + e] <-> global head 4j + 2*pr + h2
    perm = np.concatenate([
        np.arange(1024).reshape(16, 64)[[4 * j + 2 * pr + h2 for j in range(4) for h2 in range(2)]].reshape(-1)
        for pr in range(2)
    ])
    def pmajor_kt(w):
        # [C, M] -> [128(p), CT(t), M]: row t*128+p -> [p, t]
        return np.ascontiguousarray(
            w.reshape(CT, 128, w.shape[1]).transpose(1, 0, 2).astype(np.float16))

    in_maps = []
    for core in range(8):
        b, g = divmod(core, 4)
        cs = slice(g * 256, (g + 1) * 256)
        xtT = x[b].T  # [C, N]
        # xt: [QC(nch), 128(p), CT(t), QCS] with [nch,p,t,c] = xtT[t*128+p, nch*512+c]
        xt_arr = np.ascontiguousarray(
            xtT.reshape(CT, 128, QC, QCS).transpose(2, 1, 0, 3).astype(np.float16))
        # wqk blocks: 0 = q pr0, 1 = q pr1, 2 = k pr0, 3 = k pr1 (128 cols each)
        qcols = w_qkv[:, 0 * C + g * 256:0 * C + (g + 1) * 256]
        kcols = w_qkv[:, 1 * C + g * 256:1 * C + (g + 1) * 256]
        wqk = np.concatenate([qcols, kcols], axis=1)  # [C, 512]
        wqk_arr = np.ascontiguousarray(
            wqk.reshape(CT, 128, 4, 128).transpose(2, 1, 0, 3).astype(np.float16))
        in_maps.append({
            "xt": xt_arr,
            "wqk": wqk_arr,
            "wv": pmajor_kt(w_qkv[:, 2 * C + g * 256:2 * C + (g + 1) * 256]),
            "wpc": pmajor_kt(w_proj[perm, :][:, cs]),
            "bc": np.ascontiguousarray(b_proj[cs].reshape(2, 128).T),
        })
    return in_maps


def assemble_output(results):
    outT = np.empty((B, C, N), dtype=np.float32)
    for core in range(8):
        b, g = divmod(core, 4)
        outT[b, g * 256:(g + 1) * 256, :] = np.asarray(results[core]["out"], dtype=np.float32)
    return np.ascontiguousarray(outT.transpose(0, 2, 1))


def run_sharded(x, w_qkv, w_proj, b_proj, trace=False):
    nc = _get_nc()
    in_maps = shard_inputs(x, w_qkv, w_proj, b_proj)
    res = run_bass_kernel_spmd(nc, in_maps, core_ids=list(range(8)), trace=trace)
    return assemble_output(res.results), res.exec_time_ns


def kernel(x, w_qkv, w_proj, b_proj):
    out, _ = run_sharded(x, w_qkv, w_proj, b_proj, trace=False)
    return out


# revision 19
# speedup vs baseline: 1.2172x; 1.0634x over previous
"""Multi-head attention block (B=2, N=2048, C=1024, H=16, hd=64) on 8 TRN2 NeuronCores.

Sharding: data-parallel over batch (2 groups of 4 cores), tensor-parallel over
heads within each group (4 heads/core). Each core computes q/k/v for its heads,
attention, and a partial output projection; an AllGather over the 4-core group
collects head outputs, and each core projects its 256-column slice.

v2 schedule: single fused loop. The softmax exp on the Scalar engine
(~1.3us per 1024 columns x 128 iterations) and the PE matmul stream
(~164us of column-cycles) are the two near-equal rooflines, so all QKV /
projection matmuls are streamed INTO the attention loop's PE slack instead
of running in separate phases where the other engine would idle. Blocks run
pr-major ((qc,pr0) x4 then (qc,pr1) x4) so only block 0 carries forced k/v
emissions. Input DMA is a few large transfers on both HWDGE rings (SP+ACT)
ordered so the first score matmul can start ~3us in.

Per-core layouts (contraction dim on SBUF partitions; host pre-transposes x):
  xt   [1024, 2048]  x[b].T
  wqk  [1024, 512]   w_qkv columns for this core's q (256) ++ k (256)
  wv   [1024, 256]   w_qkv columns for this core's v
  wpb  [256, 1024]   w_proj rows for this core's heads (perm'd, see host code)
  bc   [128, 2]      bc[p, m] = b_proj[g*256 + m*128 + p]
  out  [256, 2048]   rows g*256:(g+1)*256 of (x[b] @ ... ).T
"""
import sys

if '/opt/trn_rl_repo' not in sys.path:
    sys.path.insert(0, '/opt/trn_rl_repo')

import numpy as np

import concourse.bass as bass
import concourse.mybir as mybir
import concourse.tile as tile
from concourse import bacc
from concourse.bass_utils import run_bass_kernel_spmd

F32 = mybir.dt.float32
F16 = mybir.dt.float16

B = 2
N = 2048          # sequence length
C = 1024          # model dim
HD = 64           # head dim
SCALE = HD ** -0.5
NT = N // 128     # 16 key tiles
CT = C // 128     # 8 contraction tiles
QC = 4            # q-chunks of 512
QCS = N // QC     # 512
GROUPS = [[0, 1, 2, 3], [4, 5, 6, 7]]
NITER = 8 * NT    # 8 blocks x 16 key tiles

_NC_CACHE = None


def _blk(b):
    """qc-major block order: b = 2*qc + pr."""
    return (b // 2, b % 2)  # (qc, pr)


def build():
    nc = bacc.Bacc(None, target_bir_lowering=False, debug=False)

    # p-major host layouts: every input DMA moves ~128 multi-KB descriptors
    # (HWDGE issue time scales with descriptor count)
    xt_ext = nc.declare_dram_parameter("xt", [QC, 128, CT, QCS], F16, isOutput=False)
    wqk_ext = nc.declare_dram_parameter("wqk", [4, 128, CT, 128], F16, isOutput=False)
    wv_ext = nc.declare_dram_parameter("wv", [128, CT, 256], F16, isOutput=False)
    wpc_ext = nc.declare_dram_parameter("wpc", [128, CT, 256], F16, isOutput=False)
    bc_ext = nc.declare_dram_parameter("bc", [128, 2], F32, isOutput=False)
    out_ext = nc.declare_dram_parameter("out", [256, N], F16, isOutput=True)

    with tile.TileContext(nc) as tc:
        with (
            tc.tile_pool(name="weights", bufs=1) as wpool,
            tc.tile_pool(name="acts", bufs=1) as apool,
            tc.tile_pool(name="expt", bufs=3) as epool,
            tc.tile_pool(name="norm", bufs=2) as npool,
            tc.tile_pool(name="outp", bufs=2) as opool,
            tc.tile_pool(name="ofp", bufs=2) as ofpool,
            tc.tile_pool(name="psS", bufs=2, space="PSUM") as psS_pool,
            tc.tile_pool(name="psE", bufs=2, space="PSUM") as psE_pool,
            tc.tile_pool(name="psO", bufs=1, space="PSUM") as psO_pool,
            tc.tile_pool(name="dramog", bufs=2, space="DRAM") as og_pool,
            tc.tile_pool(name="dramag", bufs=8, space="DRAM") as ag_pool,
        ):
            # ---- SBUF tiles ----
            # xt/wqk chunk-major so each input DMA lands in a contiguous
            # per-partition region (large descriptors)
            xt_sb = apool.tile([128, QC, CT, QCS], F16, tag="xt")
            wqk_sb = wpool.tile([128, 4, CT, 128], F16, tag="wqk")
            wv_sb = wpool.tile([128, CT, 256], F16, tag="wv")
            wp_sb = wpool.tile([128, CT, 256], F16, tag="wp")
            bc_sb = wpool.tile([128, 2], F32, tag="bc")
            ones_row = wpool.tile([1, 64], F16, tag="ones_row")
            qk_sb = apool.tile([128, 4, N], F16, tag="qk")
            v_sb = apool.tile([128, NT, 4, 128], F16, tag="v")

            # constant fills on DVE (a DMA would be thousands of descriptors);
            # v columns 65:128 are never read (PV stationary is 65 cols wide)
            nc.vector.memset(ones_row[:, :], 1.0)
            nc.vector.memset(v_sb[:, :, :, HD:HD + 1], 1.0)

            # preload the Exp activation table while input DMAs stream
            dmy = npool.tile([1, 8], F32, tag="dmy")
            nc.vector.memset(dmy[:, :], 0.0)
            dmy2 = npool.tile([1, 8], F16, tag="dmy2")
            nc.scalar.activation(dmy2[:, :], dmy[:, :],
                                 mybir.ActivationFunctionType.Exp,
                                 bias=0.0, scale=1.0)

            # ---- input DMAs: one per chunk, critical-path first, all on the
            # SP HWDGE ring (the ACT sequencer must stay free for exp).
            # wqk blocks: 0 = q pr0, 1 = q pr1, 2 = k pr0, 3 = k pr1.
            nc.sync.dma_start(out=wqk_sb[:, 2, :, :], in_=wqk_ext.ap()[2])
            nc.sync.dma_start(out=xt_sb[:, 0, :, :], in_=xt_ext.ap()[0])
            nc.sync.dma_start(out=wqk_sb[:, 0, :, :], in_=wqk_ext.ap()[0])
            nc.sync.dma_start(out=wv_sb[:, :, :], in_=wv_ext.ap())
            nc.sync.dma_start(out=xt_sb[:, 1, :, :], in_=xt_ext.ap()[1])
            nc.sync.dma_start(out=xt_sb[:, 2, :, :], in_=xt_ext.ap()[2])
            nc.sync.dma_start(out=xt_sb[:, 3, :, :], in_=xt_ext.ap()[3])
            nc.sync.dma_start(out=wqk_sb[:, 1, :, :], in_=wqk_ext.ap()[1])
            nc.sync.dma_start(out=wqk_sb[:, 3, :, :], in_=wqk_ext.ap()[3])
            nc.sync.dma_start(out=wp_sb[:, :, :], in_=wpc_ext.ap())
            nc.sync.dma_start(out=bc_sb[:, :], in_=bc_ext[:, :])

            # ---- emission helpers (PE work streamed into the loop) ----
            def xtcol(ct, kt):
                off = (kt % 4) * 128
                return xt_sb[:, kt // 4, ct, off:off + 128]

            def emit_kquad(pr, nch):
                """k for 4 key tiles at once: one stationary per ct streams a
                full 512-col xt chunk, so LDWEIGHTS hides under the matmul."""
                ksl = slice(nch * QCS, (nch + 1) * QCS)
                psq = psE_pool.tile([128, QCS], F32, tag="psE",
                                    name=f"psk_{pr}_{nch}")
                for ct in range(CT):
                    nc.tensor.matmul(
                        psq[:, :],
                        wqk_sb[:, 2 + pr, ct, :],
                        xt_sb[:, nch, ct, :],
                        start=(ct == 0), stop=(ct == CT - 1),
                    )
                nc.vector.tensor_copy(qk_sb[:, 2 + pr, ksl], psq[:, :])

            def emit_q(pr, qc):
                qsl = slice(qc * QCS, (qc + 1) * QCS)
                psq = psE_pool.tile([128, QCS], F32, tag="psE", name=f"psq_{pr}_{qc}")
                for ct in range(CT):
                    nc.tensor.matmul(
                        psq[:, :],
                        wqk_sb[:, pr, ct, :],
                        xt_sb[:, qc, ct, :],
                        start=(ct == 0), stop=(ct == CT - 1),
                    )
                nc.vector.tensor_copy(qk_sb[:, pr, qsl], psq[:, :])

            def v_mms(kt):
                """The 8 accumulation matmuls for v(kt), to be interleaved
                between long attention matmuls (hides their LDWEIGHTS)."""
                psv = psE_pool.tile([128, 256], F32, tag="psE", name=f"psv_{kt}")

                def mm(ct, psv=psv, kt=kt):
                    nc.tensor.matmul(
                        psv[:, :],
                        xtcol(ct, kt),
                        wv_sb[:, ct, :],
                        start=(ct == 0), stop=(ct == CT - 1),
                    )

                def fin(psv=psv, kt=kt):
                    nc.vector.tensor_copy(
                        v_sb[:, kt, :, 0:HD],
                        psv[:, :].rearrange("p (h e) -> p h e", h=4),
                    )
                return [lambda ct=ct: mm(ct) for ct in range(CT)], fin

            def scores(b, kt):
                qc, pr = _blk(b)
                qsl = slice(qc * QCS, (qc + 1) * QCS)
                ksl = slice(kt * 128, (kt + 1) * 128)
                psS = psS_pool.tile([128, 2 * QCS], F32, tag="psS",
                                    name=f"psS_{b}_{kt}")
                nc.tensor.matmul(
                    psS[:, 0:QCS],
                    qk_sb[0:64, 2 + pr, ksl],
                    qk_sb[0:64, pr, qsl],
                    start=True, stop=True,
                )
                nc.tensor.matmul(
                    psS[:, QCS:2 * QCS],
                    qk_sb[64:128, 2 + pr, ksl],
                    qk_sb[64:128, pr, qsl],
                    start=True, stop=True,
                )
                return psS

            # ---- per-block normalize / gather / project ----
            norm_state = {}
            ags = {}

            def part1(b):
                """Drain psO: o and rowsums to SBUF (frees psO for next block)."""
                psO = norm_state.pop(('psO', b))
                o2 = npool.tile([128, QCS], F32, tag="o2", name=f"o2_{b}")
                rs_e = npool.tile([1, QCS], F32, tag="rs_e", name=f"rse_{b}")
                rs_o = npool.tile([1, QCS], F32, tag="rs_o", name=f"rso_{b}")
                # bank A (head e) first so next block's first PV can start early
                nc.vector.tensor_copy(o2[0:64, :], psO[0:64, 0:QCS])
                nc.vector.tensor_copy(rs_e[:, :], psO[64:65, 0:QCS])
                nc.vector.tensor_copy(o2[64:128, :], psO[0:64, QCS:2 * QCS])
                nc.vector.tensor_copy(rs_o[:, :], psO[64:65, QCS:2 * QCS])
                norm_state[('o2', b)] = o2
                norm_state[('rs', b)] = (rs_e, rs_o)

            def part2(b):
                """1/rowsum (fast approx), broadcast via PE matmul, normalize."""
                o2 = norm_state.pop(('o2', b))
                rs_e, rs_o = norm_state.pop(('rs', b))
                psB = psE_pool.tile([128, QCS], F32, tag="psE", name=f"psB_{b}")
                for hh, rs in ((0, rs_e), (1, rs_o)):
                    rcf = npool.tile([1, QCS], F32, tag="rcf", name=f"rcf_{b}_{hh}")
                    nc.vector.reciprocal_approx_fast(out=rcf[:, :], in_=rs[:, :])
                    rc16 = npool.tile([1, QCS], F16, tag="rc16", name=f"rc16_{b}_{hh}")
                    nc.vector.tensor_copy(rc16[:, :], rcf[:, :])
                    nc.tensor.matmul(psB[hh * 64:(hh + 1) * 64, :],
                                     ones_row[:, :], rc16[:, :],
                                     start=True, stop=True)
                on_sb = npool.tile([128, QCS], F16, tag="on", name=f"on_{b}")
                nc.vector.tensor_mul(on_sb[:, :], o2[:, :], psB[:, :])
                norm_state[('on', b)] = on_sb

            def part3(b):
                """Store + AllGather this block's head outputs."""
                qc, pr = _blk(b)
                on_sb = norm_state.pop(('on', b))
                og = og_pool.tile([128, QCS], F16, tag="og", name=f"og_{b}")
                nc.sync.dma_start(out=og[:, :], in_=on_sb[:, :])
                ag = ag_pool.tile([512, QCS], F16, tag="ag", name=f"ag_{b}")
                nc.gpsimd.collective_compute(
                    "AllGather",
                    mybir.AluOpType.bypass,
                    replica_groups=GROUPS,
                    ins=[og.opt()],
                    outs=[ag.opt()],
                )
                ags[(qc, pr)] = ag

            of_sbs = {}

            def emit_ofload(qc):
                of_sb = ofpool.tile([128, CT, QCS], F16, tag="of", name=f"of_{qc}")
                for pr in range(2):
                    ag_r = ags[(qc, pr)][:, :].rearrange("(t p) n -> p t n", p=128)
                    nc.sync.dma_start(out=of_sb[:, pr * 4:(pr + 1) * 4, :], in_=ag_r)
                of_sbs[qc] = of_sb

            def emit_proj(qc, m2):
                qsl = slice(qc * QCS, (qc + 1) * QCS)
                of_sb = of_sbs[qc]
                psP = psE_pool.tile([128, QCS], F32, tag="psE", name=f"psP_{qc}_{m2}")
                for t in range(CT):
                    nc.tensor.matmul(
                        psP[:, :],
                        wp_sb[:, t, m2 * 128:(m2 + 1) * 128],
                        of_sb[:, t, :],
                        start=(t == 0), stop=(t == CT - 1),
                    )
                outsb = opool.tile([128, QCS], F16, tag="outsb",
                                   name=f"outsb_{qc}_{m2}")
                nc.vector.tensor_scalar_add(outsb[:, :], psP[:, :],
                                            bc_sb[:, m2:m2 + 1])
                nc.sync.dma_start(out=out_ext[m2 * 128:(m2 + 1) * 128, qsl],
                                  in_=outsb[:, :])

            # ---- static emission schedule: iter -> list of thunks ----
            sched = {}

            def at(i, fn):
                sched.setdefault(i, []).append(fn)

            at(1, lambda: emit_kquad(0, 1))       # k(pr0) due iters 4/8/12
            at(5, lambda: emit_kquad(0, 2))
            at(9, lambda: emit_kquad(0, 3))
            at(11, lambda: emit_q(1, 0))          # due block 1 (iter 15)
            at(12, lambda: emit_kquad(1, 0))      # k(pr1) due iter 16/20/24/28
            at(17, lambda: emit_kquad(1, 1))
            at(21, lambda: emit_kquad(1, 2))
            at(25, lambda: emit_kquad(1, 3))
            at(27, lambda: emit_q(0, 1))          # due block 2 (iter 31)
            at(43, lambda: emit_q(1, 1))          # due block 3 (iter 47)
            at(59, lambda: emit_q(0, 2))          # due block 4 (iter 63)
            at(75, lambda: emit_q(1, 2))          # due block 5 (iter 79)
            at(91, lambda: emit_q(0, 3))          # due block 6 (iter 95)
            at(107, lambda: emit_q(1, 3))         # due block 7 (iter 111)
            for qc in range(3):                   # proj(qc): AG(qc,1) done at
                at(32 * qc + 49, lambda qc=qc: emit_ofload(qc))   # ~iter 38+32qc
                at(32 * qc + 52, lambda qc=qc: emit_proj(qc, 0))
                at(32 * qc + 54, lambda qc=qc: emit_proj(qc, 1))
            for b in range(7):                    # normalize tail of each block
                at(16 * b + 17, lambda b=b: part2(b))
                at(16 * b + 18, lambda b=b: part3(b))

            # v(kt) streamed during block 0, matmuls interleaved between the
            # long attention matmuls so each LDWEIGHTS hides under them
            v_plan = {kt - 1: kt for kt in range(1, NT)}

            # ---- prologue ----
            emit_kquad(0, 0)
            emit_q(0, 0)
            vm, vfin = v_mms(0)
            for m in vm:
                m()
            vfin()
            psS_cur = scores(0, 0)

            # ---- main loop ----
            for bi in range(NITER):
                b, kt = bi // NT, bi % NT
                qc, pr = _blk(b)
                vkt = v_plan.get(bi)
                vm, vfin = v_mms(vkt) if vkt is not None else ([], None)
                if bi + 1 < NITER:
                    nb, nkt = (b, kt + 1) if kt < NT - 1 else (b + 1, 0)
                    nqc, npr = _blk(nb)
                    qsl_ = slice(nqc * QCS, (nqc + 1) * QCS)
                    ksl_ = slice(nkt * 128, (nkt + 1) * 128)
                    psS_next = psS_pool.tile([128, 2 * QCS], F32, tag="psS",
                                             name=f"psS_{nb}_{nkt}")
                    nc.tensor.matmul(psS_next[:, 0:QCS],
                                     qk_sb[0:64, 2 + npr, ksl_],
                                     qk_sb[0:64, npr, qsl_],
                                     start=True, stop=True)
                    for m in vm[0:2]:
                        m()
                    nc.tensor.matmul(psS_next[:, QCS:2 * QCS],
                                     qk_sb[64:128, 2 + npr, ksl_],
                                     qk_sb[64:128, npr, qsl_],
                                     start=True, stop=True)
                    for m in vm[2:4]:
                        m()
                else:
                    psS_next = None
                    for m in vm[0:4]:
                        m()
                expt = epool.tile([128, 2 * QCS], F16, tag="expt",
                                  name=f"expt_{bi}")
                nc.scalar.activation(
                    expt[:, :], psS_cur[:, :],
                    mybir.ActivationFunctionType.Exp,
                    bias=0.0, scale=SCALE,
                )
                for fn in sched.get(bi, ()):
                    fn()
                if kt == 0:
                    norm_state[('psO', b)] = psO_pool.tile(
                        [128, 2 * QCS], F32, tag="psO", name=f"psO_{b}")
                psO = norm_state[('psO', b)]
                nc.tensor.matmul(
                    psO[0:65, 0:QCS],
                    v_sb[:, kt, 2 * pr, 0:65],
                    expt[:, 0:QCS],
                    start=(kt == 0), stop=(kt == NT - 1),
                )
                for m in vm[4:6]:
                    m()
                nc.tensor.matmul(
                    psO[0:65, QCS:2 * QCS],
                    v_sb[:, kt, 2 * pr + 1, 0:65],
                    expt[:, QCS:2 * QCS],
                    start=(kt == 0), stop=(kt == NT - 1),
                )
                for m in vm[6:8]:
                    m()
                if vfin is not None:
                    vfin()
                psS_cur = psS_next
                if kt == NT - 1:
                    part1(b)

            # ---- tail: last block's normalize + gather + projection ----
            part2(7)
            part3(7)
            emit_ofload(3)
            emit_proj(3, 0)
            emit_proj(3, 1)

    nc.compile()
    return nc


def _get_nc():
    global _NC_CACHE
    if _NC_CACHE is None:
        _NC_CACHE = build()
    return _NC_CACHE


def shard_inputs(x, w_qkv, w_proj, b_proj):
    x = np.asarray(x, dtype=np.float32)
    w_qkv = np.asarray(w_qkv, dtype=np.float32)
    w_proj = np.asarray(w_proj, dtype=np.float32)
    b_proj = np.asarray(b_proj, dtype=np.float32)
    # ag row order: for each pr, rank-major then local-head-major:
    # rows [j*128 + h2*64 + e] <-> global head 4j + 2*pr + h2
    perm = np.concatenate([
        np.arange(1024).reshape(16, 64)[[4 * j + 2 * pr + h2 for j in range(4) for h2 in range(2)]].reshape(-1)
        for pr in range(2)
    ])
    def pmajor_kt(w):
        # [C, M] -> [128(p), CT(t), M]: row t*128+p -> [p, t]
        return np.ascontiguousarray(
            w.reshape(CT, 128, w.shape[1]).transpose(1, 0, 2).astype(np.float16))

    in_maps = []
    for core in range(8):
        b, g = divmod(core, 4)
        cs = slice(g * 256, (g + 1) * 256)
        xtT = x[b].T  # [C, N]
        # xt: [QC(nch), 128(p), CT(t), QCS] with [nch,p,t,c] = xtT[t*128+p, nch*512+c]
        xt_arr = np.ascontiguousarray(
            xtT.reshape(CT, 128, QC, QCS).transpose(2, 1, 0, 3).astype(np.float16))
        # wqk blocks: 0 = q pr0, 1 = q pr1, 2 = k pr0, 3 = k pr1 (128 cols each)
        qcols = w_qkv[:, 0 * C + g * 256:0 * C + (g + 1) * 256]
        kcols = w_qkv[:, 1 * C + g * 256:1 * C + (g + 1) * 256]
        wqk = np.concatenate([qcols, kcols], axis=1)  # [C, 512]
        wqk_arr = np.ascontiguousarray(
            wqk.reshape(CT, 128, 4, 128).transpose(2, 1, 0, 3).astype(np.float16))
        in_maps.append({
            "xt": xt_arr,
            "wqk": wqk_arr,
            "wv": pmajor_kt(w_qkv[:, 2 * C + g * 256:2 * C + (g + 1) * 256]),
            "wpc": pmajor_kt(w_proj[perm, :][:, cs]),
            "bc": np.ascontiguousarray(b_proj[cs].reshape(2, 128).T),
        })
    return in_maps


def assemble_output(results):
    outT = np.empty((B, C, N), dtype=np.float32)
    for core in range(8):
        b, g = divmod(core, 4)
        outT[b, g * 256:(g + 1) * 256, :] = np.asarray(results[core]["out"], dtype=np.float32)
    return np.ascontiguousarray(outT.transpose(0, 2, 1))


def run_sharded(x, w_qkv, w_proj, b_proj, trace=False):
    nc = _get_nc()
    in_maps = shard_inputs(x, w_qkv, w_proj, b_proj)
    res = run_bass_kernel_spmd(nc, in_maps, core_ids=list(range(8)), trace=trace)
    return assemble_output(res.results), res.exec_time_ns


def kernel(x, w_qkv, w_proj, b_proj):
    out, _ = run_sharded(x, w_qkv, w_proj, b_proj, trace=False)
    return out
